# revision 1
# baseline (speedup 1.0000x reference)
"""CosRec-style pairwise-MLP recommender kernel for 8 Trainium2 NeuronCores.

Reference computation (per batch element b, L=32, D=64, FC=100):
    embs   = item_emb[seq_var]                      [B, L, D]
    A      = embs @ Wa^T  (Wa = W1[:, :D])          [B, L, FC]
    Bm     = embs @ Wb^T  (Wb = W1[:, D:])          [B, L, FC]
    h1     = relu(A[:,None,:,:] + Bm[:,:,None,:] + b1)   [B, L, L, FC]
    h2     = relu(h1 @ Wf2^T + bf2)                 [B, L, L, FC]
    x      = h2.sum((1, 2))                         [B, FC]
    out[b,t] = b2[item_var[b,t]] + W2[item_var[b,t]] . cat(x[b], user_emb[user_var[b]])

Strategy: data-parallel over batch (64 examples/core).  All gathers are done
on-device with indirect DMA; the [64, 32, 32, 100] per-core h tensor never
touches HBM — it lives tile-by-tile in SBUF/PSUM.  Per example:
  DVE   : pre = A'[:,c] + Bm[:,a]  (outer sum via broadcast APs, [100, 1024])
  DVE/ACT: relu in-place
  PE    : h2 = Wf2T.T @ h1 into PSUM (2 x N=512 matmuls, fp32r by default)
  ACT   : relu(h2 + bf2) with fused accumulate -> x[:, b]  (one instruction)
Final stage: per-target-t fused multiply-reduce on DVE against gathered W2
rows, with b2 as the reduction seed.
"""

import os
import sys

import numpy as np

sys.path.insert(0, "/opt/trn_rl_repo")

import concourse.bass as bass
import concourse.tile as tile
from concourse import bacc, mybir
from concourse.bass_utils import run_bass_kernel_spmd
from concourse.masks import make_identity
from contextlib import ExitStack

N_CORES = 8
B_FULL = 512
BPC = B_FULL // N_CORES  # 64 examples per core
L = 32
D = 64
FC = 100
T = 3
NROW = BPC * L           # 2048 gathered rows per core
NTILE = NROW // 128      # 16 gather tiles
F32 = mybir.dt.float32
I32 = mybir.dt.int32

# ---- tunables -------------------------------------------------------------
CFG = dict(
    r1_gps_num=16,     # of every 16 examples: relu1 on GpSimd ...
    r1_act_num=0,      # ... then this many on ScalarE; remainder on VectorE
    p3_dve_num=0,      # of every 16 examples, how many run relu2+accum on DVE (rest ACT)
    l2_f32r=True,      # layer-2 matmul in fp32r (4x faster PE, TF32-like precision)
    l1_f32r=False,     # layer-1 matmul in fp32r
    h2_bufs=2,         # PSUM double-buffering for the [100, 1024] h2 tile
    pre_bufs=4,
)

_PROG_CACHE = {}


def _build_program(cfg):
    nc = bacc.Bacc()

    seq_idx = nc.dram_tensor("seq_idx", [128, NTILE], I32, kind="ExternalInput")
    user_idx = nc.dram_tensor("user_idx", [BPC, 1], I32, kind="ExternalInput")
    item_idx = nc.dram_tensor("item_idx", [BPC, T], I32, kind="ExternalInput")
    item_emb = nc.dram_tensor("item_emb", [100000, D], F32, kind="ExternalInput")
    user_emb = nc.dram_tensor("user_emb", [100000, D], F32, kind="ExternalInput")
    W2 = nc.dram_tensor("W2", [100000, FC + D], F32, kind="ExternalInput")
    b2 = nc.dram_tensor("b2", [100000, 1], F32, kind="ExternalInput")
    W1 = nc.dram_tensor("W1", [FC, 2 * D], F32, kind="ExternalInput")
    b1 = nc.dram_tensor("b1", [FC, 1], F32, kind="ExternalInput")
    Wf2 = nc.dram_tensor("Wf2", [FC, FC], F32, kind="ExternalInput")
    bf2 = nc.dram_tensor("bf2", [FC, 1], F32, kind="ExternalInput")
    out_d = nc.dram_tensor("out", [BPC, T], F32, kind="ExternalOutput")

    Relu = mybir.ActivationFunctionType.Relu
    Add = mybir.AluOpType.add
    Mult = mybir.AluOpType.mult
    Max = mybir.AluOpType.max

    def _r1_engine(i):
        j = i % 16
        if j < cfg["r1_gps_num"]:
            return "g"
        if j < cfg["r1_gps_num"] + cfg["r1_act_num"]:
            return "a"
        return "v"

    r1_eng = [_r1_engine(i) for i in range(BPC)]
    p3_dve = [(i % 16) < cfg["p3_dve_num"] for i in range(BPC)]

    with ExitStack() as ctx:
        tc = ctx.enter_context(tile.TileContext(nc))
        const = ctx.enter_context(tc.tile_pool(name="const", bufs=1))
        gat = ctx.enter_context(tc.tile_pool(name="gat", bufs=4))
        prep = ctx.enter_context(tc.tile_pool(name="pre", bufs=cfg["pre_bufs"]))
        scrp = ctx.enter_context(tc.tile_pool(name="scr", bufs=2))
        ps_m = ctx.enter_context(tc.tile_pool(name="psm", bufs=2, space="PSUM"))
        ps_h = ctx.enter_context(
            tc.tile_pool(name="psh", bufs=cfg["h2_bufs"], space="PSUM")
        )

        # ---------------- constants & weights ----------------
        ident = const.tile([128, 128], F32)
        make_identity(nc, ident[:])

        w1_sb = const.tile([FC, 2 * D], F32)
        nc.sync.dma_start(out=w1_sb[:], in_=W1[:, :])
        wf2_sb = const.tile([FC, FC], F32)
        nc.sync.dma_start(out=wf2_sb[:], in_=Wf2[:, :])
        b1_sb = const.tile([FC, 1], F32)
        nc.sync.dma_start(out=b1_sb[:], in_=b1[:, :])
        bf2_sb = const.tile([FC, 1], F32)
        nc.sync.dma_start(out=bf2_sb[:], in_=bf2[:, :])
        idx_sb = const.tile([128, NTILE], I32)
        nc.sync.dma_start(out=idx_sb[:], in_=seq_idx[:, :])
        uidx_sb = const.tile([BPC, 1], I32)
        nc.sync.dma_start(out=uidx_sb[:], in_=user_idx[:, :])
        iidx_sb = const.tile([BPC, T], I32)
        nc.sync.dma_start(out=iidx_sb[:], in_=item_idx[:, :])

        # WaT/WbT: [64, 100] = (W1[:, :D]).T and (W1[:, D:]).T, both at base partition 0
        waT = const.tile([D, FC], F32)
        wbT = const.tile([D, FC], F32)
        for half, dst in ((0, waT), (1, wbT)):
            w1h_ps = ps_m.tile([D, FC], F32, tag="m")
            nc.tensor.transpose(
                w1h_ps[:], w1_sb[:, half * D : (half + 1) * D], ident[:FC, :FC]
            )
            nc.vector.tensor_copy(dst[:], w1h_ps[:])

        # Wf2T: [100, 100] = Wf2.T  (fp32r-rounded when layer-2 runs in fp32r)
        l2dt = mybir.dt.float32r if cfg["l2_f32r"] else F32
        wf2t_ps = ps_m.tile([FC, FC], F32, tag="m")
        nc.tensor.transpose(wf2t_ps[:], wf2_sb[:], ident[:FC, :FC])
        wf2t = const.tile([FC, FC], l2dt)
        nc.vector.tensor_copy(wf2t[:], wf2t_ps[:])

        # ---------------- embedding gather + transpose ----------------
        # embsT[d, b*L + l] = item_emb[seq[b, l], d]
        embsT = const.tile([D, NROW], F32)
        for t in range(NTILE):
            g = gat.tile([128, D], F32)
            nc.gpsimd.indirect_dma_start(
                out=g[:],
                out_offset=None,
                in_=item_emb[:, :],
                in_offset=bass.IndirectOffsetOnAxis(ap=idx_sb[:, t : t + 1], axis=0),
            )
            tp = ps_m.tile([D, 128], F32, tag="m")
            nc.tensor.transpose(tp[:], g[:], ident[:, :])
            nc.scalar.copy(embsT[:, t * 128 : (t + 1) * 128], tp[:])

        # ---------------- layer-1: A' = embs@Wa^T + b1, Bm = embs@Wb^T ------
        # A_t/Bm_t: [100(f), 2048(b*L+l)]
        A_t = const.tile([FC, NROW], F32)
        Bm_t = const.tile([FC, NROW], F32)
        l1dt = mybir.dt.float32r if cfg["l1_f32r"] else F32
        for j in range(NROW // 512):
            sl = slice(j * 512, (j + 1) * 512)
            pa = ps_m.tile([FC, 512], F32, tag="m")
            nc.tensor.matmul(
                pa[:],
                lhsT=waT[:].bitcast(l1dt),
                rhs=embsT[:, sl].bitcast(l1dt),
                start=True,
                stop=True,
            )
            # PSUM->SBUF copy with fused +b1 (per-partition bias)
            nc.scalar.activation(
                A_t[:, sl], pa[:], mybir.ActivationFunctionType.Identity,
                bias=b1_sb[:, 0:1],
            )
            pb = ps_m.tile([FC, 512], F32, tag="m")
            nc.tensor.matmul(
                pb[:],
                lhsT=wbT[:].bitcast(l1dt),
                rhs=embsT[:, sl].bitcast(l1dt),
                start=True,
                stop=True,
            )
            nc.scalar.copy(Bm_t[:, sl], pb[:])

        # ---------------- main loop over examples ----------------
        x = const.tile([FC, BPC], F32)  # x[:, b] = sum_{a,c} h2[b, a, c, :]
        for b in range(BPC):
            sl = slice(b * L, (b + 1) * L)
            pre = prep.tile([FC, L * L], l2dt)
            # pre[f, a*L + c] = A'[f, b*L + c] + Bm[f, b*L + a]
            in0 = A_t[:, sl].unsqueeze(1).to_broadcast([FC, L, L])
            in1 = Bm_t[:, sl].unsqueeze(2).to_broadcast([FC, L, L])
            nc.vector.tensor_tensor(
                out=pre[:].rearrange("p (a c) -> p a c", a=L),
                in0=in0,
                in1=in1,
                op=Add,
            )
            # relu in place
            if r1_eng[b] == "g":
                nc.gpsimd.tensor_scalar_max(pre[:], pre[:], 0.0)
            elif r1_eng[b] == "a":
                nc.scalar.activation(pre[:], pre[:], Relu)
            else:
                nc.vector.tensor_scalar_max(pre[:], pre[:], 0.0)
            # layer 2: h2pre = Wf2T.T @ h1  (PSUM, 2 banks)
            h2p = ps_h.tile([FC, L * L], F32, tag="h2")
            for half in range(2):
                hs = slice(half * 512, (half + 1) * 512)
                nc.tensor.matmul(
                    h2p[:, hs],
                    lhsT=wf2t[:],
                    rhs=pre[:, hs],
                    start=True,
                    stop=True,
                )
            # relu(h2pre + bf2), accumulate over the 1024 pairs -> x[:, b]
            h2s = scrp.tile([FC, L * L], F32, tag="h2s")
            if p3_dve[b]:
                nc.vector.tensor_scalar(
                    out=h2s[:],
                    in0=h2p[:],
                    scalar1=bf2_sb[:, 0:1],
                    scalar2=0.0,
                    op0=Add,
                    op1=Max,
                    accum_out=x[:, b : b + 1],
                )
            else:
                nc.scalar.activation(
                    h2s[:], h2p[:], Relu,
                    bias=bf2_sb[:, 0:1],
                    accum_out=x[:, b : b + 1],
                )

        # ---------------- gathers for the final stage ----------------
        # (emitted after the main loop so they don't delay it on GpSimd)
        ug = const.tile([BPC, D], F32)
        nc.gpsimd.indirect_dma_start(
            out=ug[:],
            out_offset=None,
            in_=user_emb[:, :],
            in_offset=bass.IndirectOffsetOnAxis(ap=uidx_sb[:, 0:1], axis=0),
        )
        w2g = []
        for t in range(T):
            w2g_t = const.tile([BPC, FC + D], F32, tag=f"w2g{t}")
            nc.gpsimd.indirect_dma_start(
                out=w2g_t[:],
                out_offset=None,
                in_=W2[:, :],
                in_offset=bass.IndirectOffsetOnAxis(ap=iidx_sb[:, t : t + 1], axis=0),
            )
            w2g.append(w2g_t)
        b2g = const.tile([BPC, T], F32)
        for t in range(T):
            nc.gpsimd.indirect_dma_start(
                out=b2g[:, t : t + 1],
                out_offset=None,
                in_=b2[:, :],
                in_offset=bass.IndirectOffsetOnAxis(ap=iidx_sb[:, t : t + 1], axis=0),
            )

        # ---------------- final: out[b, t] = b2 + W2row . cat(x, uemb) ------
        xT_ps = ps_m.tile([BPC, FC], F32, tag="m")
        nc.tensor.transpose(xT_ps[:], x[:], ident[:FC, :FC])
        xT = const.tile([BPC, FC], F32)
        nc.vector.tensor_copy(xT[:], xT_ps[:])

        # (tensor_tensor_reduce is broken on this runtime — use mult + accum)
        out_sb = const.tile([BPC, T], F32)
        for t in range(T):
            scr = scrp.tile([BPC, FC + D], F32, tag="fin")
            nc.vector.tensor_tensor(
                out=scr[:, 0:FC], in0=w2g[t][:, 0:FC], in1=xT[:], op=Mult
            )
            nc.vector.tensor_tensor(
                out=scr[:, FC:], in0=w2g[t][:, FC:], in1=ug[:], op=Mult
            )
            acc = scrp.tile([BPC, 1], F32, tag="facc")
            dummy = scrp.tile([BPC, FC + D], F32, tag="fdum")
            nc.scalar.activation(
                dummy[:], scr[:], mybir.ActivationFunctionType.Identity,
                accum_out=acc[:],
            )
            nc.vector.tensor_tensor(
                out=out_sb[:, t : t + 1], in0=acc[:], in1=b2g[:, t : t + 1], op=Add
            )
        nc.sync.dma_start(out=out_d[:, :], in_=out_sb[:])

    nc.finalize()
    return nc


def get_program(cfg=None):
    cfg = dict(CFG if cfg is None else cfg)
    key = tuple(sorted(cfg.items()))
    if key not in _PROG_CACHE:
        _PROG_CACHE[key] = _build_program(cfg)
    return _PROG_CACHE[key]


def make_in_maps(inputs):
    """Shard the full-problem inputs into 8 per-core input maps."""
    seq = np.asarray(inputs["seq_var"]).astype(np.int32)
    usr = np.asarray(inputs["user_var"]).astype(np.int32).reshape(B_FULL, 1)
    itm = np.asarray(inputs["item_var"]).astype(np.int32).reshape(B_FULL, T)
    shared = dict(
        item_emb=np.ascontiguousarray(np.asarray(inputs["item_emb"], np.float32)),
        user_emb=np.ascontiguousarray(np.asarray(inputs["user_emb"], np.float32)),
        W2=np.ascontiguousarray(np.asarray(inputs["W2"], np.float32)),
        b2=np.ascontiguousarray(np.asarray(inputs["b2"], np.float32).reshape(-1, 1)),
        W1=np.ascontiguousarray(np.asarray(inputs["W1"], np.float32)),
        b1=np.ascontiguousarray(np.asarray(inputs["b1"], np.float32).reshape(FC, 1)),
        Wf2=np.ascontiguousarray(np.asarray(inputs["Wf2"], np.float32)),
        bf2=np.ascontiguousarray(np.asarray(inputs["bf2"], np.float32).reshape(FC, 1)),
    )
    in_maps = []
    for c in range(N_CORES):
        rows = slice(c * BPC, (c + 1) * BPC)
        flat = seq[rows].reshape(NROW)               # (b*L + l) order
        seq_pm = np.ascontiguousarray(flat.reshape(NTILE, 128).T)  # [128, 16]
        in_maps.append(
            dict(
                shared,
                seq_idx=seq_pm,
                user_idx=np.ascontiguousarray(usr[rows]),
                item_idx=np.ascontiguousarray(itm[rows]),
            )
        )
    return in_maps


def run_sharded(inputs, cfg=None, trace=False, **kwargs):
    nc = get_program(cfg)
    in_maps = make_in_maps(inputs)
    res = run_bass_kernel_spmd(nc, in_maps, list(range(N_CORES)), trace=trace, **kwargs)
    out = np.concatenate([r["out"] for r in res.results], axis=0)
    return out, res


def kernel(**inputs) -> np.ndarray:
    out, _ = run_sharded(inputs)
    return out



# revision 24
# speedup vs baseline: 6.6118x; 6.6118x over previous
"""CosRec-style pairwise-MLP recommender kernel for 8 Trainium2 NeuronCores.

Reference computation (per batch element b, L=32, D=64, FC=100):
    embs   = item_emb[seq_var]                      [B, L, D]
    A      = embs @ Wa^T  (Wa = W1[:, :D])          [B, L, FC]
    Bm     = embs @ Wb^T  (Wb = W1[:, D:])          [B, L, FC]
    h1     = relu(A[:,None,:,:] + Bm[:,:,None,:] + b1)   [B, L, L, FC]
    h2     = relu(h1 @ Wf2^T + bf2)                 [B, L, L, FC]
    x      = h2.sum((1, 2))                         [B, FC]
    out[b,t] = b2[item_var[b,t]] + W2[item_var[b,t]] . cat(x[b], user_emb[user_var[b]])

Strategy: data-parallel over batch (64 examples/core); all gathers on-device
via indirect DMA.  The per-example [100, 1024] pair tensor lives in SBUF/PSUM
only.  Engine assignment per example (cfg-tunable):
  P1 pre-add  : DVE tensor_tensor broadcast add (fp16 out), or GpSimd
  R1 relu1    : DVE tensor_scalar_max in place (fp16 -> 4x mode), or ACT
  L2 matmul   : PE, wf2t fp16 stationary, 2 x 512-col matmuls into PSUM
  P3 relu2+acc: ACT activation(Relu, bias, accum_out) in place on PSUM, or DVE
"""

import os
import sys

import numpy as np

sys.path.insert(0, "/opt/trn_rl_repo")

import concourse.bass as bass
import concourse.tile as tile
from concourse import bacc, mybir
from concourse.bass_utils import run_bass_kernel_spmd
from concourse.masks import make_identity
from contextlib import ExitStack

N_CORES = 8
B_FULL = 512
BPC = B_FULL // N_CORES  # 64 examples per core
L = 32
D = 64
FC = 100
T = 3
NROW = BPC * L           # 2048 gathered rows per core
NTILE = NROW // 128      # 16 gather tiles
NCHUNK = 4               # layer-1 512-col chunks
F32 = mybir.dt.float32
F16 = mybir.dt.float16
I32 = mybir.dt.int32

# ---- tunables -------------------------------------------------------------
CFG = dict(
    p1_batch=2,     # examples per DVE pre-add/relu1 instruction (divides 16)
    p1_gps_lo=0,    # of every 16//p1_batch groups with b < 32: P1 on GpSimd
    p1_gps_hi=0,    # of every 16//p1_batch groups with b >= 32: P1 on GpSimd
    r1_act_num=0,   # of every 16//p1_batch groups: relu1 on ACT instead of DVE
    p3_dve_num=0,   # of every 16 examples: relu2+accum on DVE instead of ACT
    p3_inplace=True,   # ACT relu2 writes back into the PSUM tile
    # fp16 pre/L2 fails the 2e-2 gate: Wf2 quantized to fp16 alone costs
    # 2.2e-2 max-rel.  f32r stores full fp32; the PE's internal rounding is
    # much gentler (baseline measured 6.8e-3).
    pre_f16=False,     # pre/h1 + layer-2 in fp16 (DVE 4x relu); else fp32r
    l1_f16=True,       # layer-1 matmul in fp16 (else fp32 4-pass)
    mm_1024=False,     # single 1024-col layer-2 matmul: ILLEGAL (ISA check)
    pre_bufs=4,
    h2_bufs=2,
)

_PROG_CACHE = {}


def _build_program(cfg):
    nc = bacc.Bacc()

    seq_idx = nc.dram_tensor("seq_idx", [128, NTILE], I32, kind="ExternalInput")
    user_idx = nc.dram_tensor("user_idx", [BPC, 1], I32, kind="ExternalInput")
    item_idx = nc.dram_tensor("item_idx", [BPC, T], I32, kind="ExternalInput")
    item_emb = nc.dram_tensor("item_emb", [100000, D], F32, kind="ExternalInput")
    user_emb = nc.dram_tensor("user_emb", [100000, D], F32, kind="ExternalInput")
    W2 = nc.dram_tensor("W2", [100000, FC + D], F32, kind="ExternalInput")
    b2 = nc.dram_tensor("b2", [100000, 1], F32, kind="ExternalInput")
    W1 = nc.dram_tensor("W1", [FC, 2 * D], F32, kind="ExternalInput")
    b1 = nc.dram_tensor("b1", [FC, 1], F32, kind="ExternalInput")
    Wf2 = nc.dram_tensor("Wf2", [FC, FC], F32, kind="ExternalInput")
    bf2 = nc.dram_tensor("bf2", [FC, 1], F32, kind="ExternalInput")
    out_d = nc.dram_tensor("out", [BPC, T], F32, kind="ExternalOutput")

    Relu = mybir.ActivationFunctionType.Relu
    Ident = mybir.ActivationFunctionType.Identity
    Add = mybir.AluOpType.add
    Mult = mybir.AluOpType.mult
    Max = mybir.AluOpType.max

    # pre/h1 and the layer-2 matmul share a dtype: fp16, or fp32r (1-pass PE
    # either way; f32r tiles are written rounded, satisfying the verifier)
    predt = F16 if cfg["pre_f16"] else mybir.dt.float32r
    # layer-1 in fp16 (1-pass PE) or fp32 (4-pass); fp32r would require
    # f32r-rounded producers for embsT/waT/wbT
    l1dt = F16 if cfg["l1_f16"] else F32

    def _p1_gps(b0):
        j = (b0 % 16) // cfg["p1_batch"]
        return j < (cfg["p1_gps_lo"] if b0 < 32 else cfg["p1_gps_hi"])

    def _r1_act(b0):
        ngrp = 16 // cfg["p1_batch"]
        return ((b0 % 16) // cfg["p1_batch"]) >= ngrp - cfg["r1_act_num"]

    def _p3_dve(b):
        # offset by 8 so p3-DVE examples interleave with r1-ACT ones
        return ((b + 8) % 16) < cfg["p3_dve_num"]

    with ExitStack() as ctx:
        tc = ctx.enter_context(tile.TileContext(nc))
        const = ctx.enter_context(tc.tile_pool(name="const", bufs=1))
        gat = ctx.enter_context(tc.tile_pool(name="gat", bufs=4))
        prep = ctx.enter_context(tc.tile_pool(name="pre", bufs=cfg["pre_bufs"]))
        scrp = ctx.enter_context(tc.tile_pool(name="scr", bufs=2))
        ps_m = ctx.enter_context(tc.tile_pool(name="psm", bufs=2, space="PSUM"))
        ps_h = ctx.enter_context(
            tc.tile_pool(name="psh", bufs=cfg["h2_bufs"], space="PSUM")
        )

        # ---------------- input DMAs (seq_idx first: gathers depend on it) --
        idx_sb = const.tile([128, NTILE], I32)
        nc.sync.dma_start(out=idx_sb[:], in_=seq_idx[:, :])
        uidx_sb = const.tile([BPC, 1], I32)
        nc.sync.dma_start(out=uidx_sb[:], in_=user_idx[:, :])
        iidx_sb = const.tile([BPC, T], I32)
        nc.sync.dma_start(out=iidx_sb[:], in_=item_idx[:, :])
        w1_sb = const.tile([FC, 2 * D], F32)
        nc.sync.dma_start(out=w1_sb[:], in_=W1[:, :])
        wf2_sb = const.tile([FC, FC], F32)
        nc.sync.dma_start(out=wf2_sb[:], in_=Wf2[:, :])
        b1_sb = const.tile([FC, 1], F32)
        nc.sync.dma_start(out=b1_sb[:], in_=b1[:, :])
        bf2_sb = const.tile([FC, 1], F32)
        nc.sync.dma_start(out=bf2_sb[:], in_=bf2[:, :])

        # ---------------- embedding gathers (GpSimd queue, early) -----------
        gtiles = []
        for t in range(NTILE):
            g = gat.tile([128, D], F32, tag=f"g{t % 4}")
            nc.gpsimd.indirect_dma_start(
                out=g[:],
                out_offset=None,
                in_=item_emb[:, :],
                in_offset=bass.IndirectOffsetOnAxis(ap=idx_sb[:, t : t + 1], axis=0),
            )
            gtiles.append(g)

        # final-stage gather tiles (DMAs issued late in the main loop: issuing
        # them early puts a framework DRAIN on the dynamic-DMA queue whose
        # event consumers are in the tail, stalling the queue for the whole
        # main loop)
        xcat = const.tile([BPC, FC + D], F32)   # [x | user_emb] per example
        w2g = [
            const.tile([BPC, FC + D], F32, name=f"w2g{t}", tag=f"w2g{t}")
            for t in range(T)
        ]
        b2g = const.tile([BPC, T], F32)

        def emit_final_gathers():
            nc.gpsimd.indirect_dma_start(
                out=xcat[:, FC:],
                out_offset=None,
                in_=user_emb[:, :],
                in_offset=bass.IndirectOffsetOnAxis(ap=uidx_sb[:, 0:1], axis=0),
            )
            for t in range(T):
                nc.gpsimd.indirect_dma_start(
                    out=w2g[t][:],
                    out_offset=None,
                    in_=W2[:, :],
                    in_offset=bass.IndirectOffsetOnAxis(
                        ap=iidx_sb[:, t : t + 1], axis=0
                    ),
                )
            for t in range(T):
                nc.gpsimd.indirect_dma_start(
                    out=b2g[:, t : t + 1],
                    out_offset=None,
                    in_=b2[:, :],
                    in_offset=bass.IndirectOffsetOnAxis(
                        ap=iidx_sb[:, t : t + 1], axis=0
                    ),
                )

        # ---------------- constants & weights ----------------
        ident = const.tile([128, 128], F32)
        make_identity(nc, ident[:])

        # WaT/WbT: [64, 100] = (W1[:, :D]).T and (W1[:, D:]).T
        waT = const.tile([D, FC], l1dt)
        wbT = const.tile([D, FC], l1dt)
        for half, dst in ((0, waT), (1, wbT)):
            w1h_ps = ps_m.tile([D, FC], F32, tag="m")
            nc.tensor.transpose(
                w1h_ps[:], w1_sb[:, half * D : (half + 1) * D], ident[:FC, :FC]
            )
            nc.vector.tensor_copy(dst[:], w1h_ps[:])

        # Wf2T: [100, 100] = Wf2.T in the layer-2 matmul dtype
        wf2t_ps = ps_m.tile([FC, FC], F32, tag="m")
        nc.tensor.transpose(wf2t_ps[:], wf2_sb[:], ident[:FC, :FC])
        wf2t = const.tile([FC, FC], predt)
        nc.vector.tensor_copy(wf2t[:], wf2t_ps[:])

        # ------- embsT + layer-1, interleaved per 512-col chunk -------------
        # embsT[d, b*L + l] = item_emb[seq[b, l], d]
        # A_t/Bm_t: [100(f), 2048(b*L+l)],  A_t includes +b1
        embsT = const.tile([D, NROW], l1dt)
        A_t = const.tile([FC, NROW], F32)
        Bm_t = const.tile([FC, NROW], F32)
        for j in range(NCHUNK):
            for t in range(4 * j, 4 * j + 4):
                tp = ps_m.tile([D, 128], F32, tag="m")
                nc.tensor.transpose(tp[:], gtiles[t][:], ident[:, :])
                if t % 2 == 0:
                    nc.scalar.copy(embsT[:, t * 128 : (t + 1) * 128], tp[:])
                else:
                    nc.vector.tensor_copy(embsT[:, t * 128 : (t + 1) * 128], tp[:])
            sl = slice(j * 512, (j + 1) * 512)
            # NOTE: b1 is NOT folded into A_t.  A/B are ~10x smaller than b1,
            # so a pre-add that includes b1 loses a decimal digit of the
            # signal when written fp16.  b1 is applied inside relu1 instead
            # (fp32 scalar path on DVE/ACT).
            pa = ps_m.tile([FC, 512], F32, tag="m")
            nc.tensor.matmul(
                pa[:], lhsT=waT[:], rhs=embsT[:, sl], start=True, stop=True
            )
            nc.scalar.copy(A_t[:, sl], pa[:])
            pb = ps_m.tile([FC, 512], F32, tag="m")
            nc.tensor.matmul(
                pb[:], lhsT=wbT[:], rhs=embsT[:, sl], start=True, stop=True
            )
            nc.vector.tensor_copy(Bm_t[:, sl], pb[:])

        # ---------------- main loop over examples ----------------
        # Examples are processed in pairs: one DVE pre-add + one relu1 per
        # pair (4D broadcast APs), halving per-instruction overhead.
        # relu1 applies b1 (per-partition fp32 scalar) and the relu.
        x = const.tile([FC, BPC], F32)  # x[:, b] = sum_{a,c} h2[b, a, c, :]
        EB = cfg["p1_batch"]            # examples per P1/R1 instruction
        for b0 in range(0, BPC, EB):
            bs = list(range(b0, b0 + EB))
            sl = slice(b0 * L, (b0 + EB) * L)
            pre = prep.tile([FC, EB * L * L], predt)
            # pre[f, (e, a, c)] = A[f, e*L + c] + Bm[f, e*L + a]
            in0 = (
                A_t[:, sl]
                .rearrange("p (e c) -> p e c", e=EB)
                .unsqueeze(2)
                .to_broadcast([FC, EB, L, L])
            )
            in1 = (
                Bm_t[:, sl]
                .rearrange("p (e a) -> p e a", e=EB)
                .unsqueeze(3)
                .to_broadcast([FC, EB, L, L])
            )
            p1eng = nc.gpsimd if _p1_gps(b0) else nc.vector
            p1eng.tensor_tensor(
                out=pre[:].rearrange("p (e a c) -> p e a c", e=EB, a=L),
                in0=in0,
                in1=in1,
                op=Add,
            )
            # relu1 in place: max(pre + b1, 0)   (b1 in fp32 via scalar path)
            if _r1_act(b0):
                nc.scalar.activation(pre[:], pre[:], Relu, bias=b1_sb[:, 0:1])
            else:
                nc.vector.tensor_scalar(
                    out=pre[:],
                    in0=pre[:],
                    scalar1=b1_sb[:, 0:1],
                    scalar2=0.0,
                    op0=Add,
                    op1=Max,
                )
            for b in bs:
                # layer 2: h2pre = Wf2T.T @ h1  (PSUM, 2 banks)
                h2p = ps_h.tile([FC, L * L], F32, tag="h2")
                boff = (b - b0) * L * L
                if cfg["mm_1024"]:
                    nc.tensor.matmul(
                        h2p[:],
                        lhsT=wf2t[:],
                        rhs=pre[:, boff : boff + L * L],
                        start=True,
                        stop=True,
                    )
                else:
                    for half in range(2):
                        nc.tensor.matmul(
                            h2p[:, half * 512 : (half + 1) * 512],
                            lhsT=wf2t[:],
                            rhs=pre[:, boff + half * 512 : boff + (half + 1) * 512],
                            start=True,
                            stop=True,
                        )
                # relu(h2pre + bf2), accumulate the 1024 pairs -> x[:, b]
                if _p3_dve(b):
                    h2s = scrp.tile([FC, L * L], predt, tag="h2s")
                    nc.vector.tensor_scalar(
                        out=h2s[:],
                        in0=h2p[:],
                        scalar1=bf2_sb[:, 0:1],
                        scalar2=0.0,
                        op0=Add,
                        op1=Max,
                        accum_out=x[:, b : b + 1],
                    )
                elif cfg["p3_inplace"]:
                    nc.scalar.activation(
                        h2p[:], h2p[:], Relu,
                        bias=bf2_sb[:, 0:1],
                        accum_out=x[:, b : b + 1],
                    )
                else:
                    h2s = scrp.tile([FC, L * L], predt, tag="h2s")
                    nc.scalar.activation(
                        h2s[:], h2p[:], Relu,
                        bias=bf2_sb[:, 0:1],
                        accum_out=x[:, b : b + 1],
                    )
            if b0 + EB == 48:
                emit_final_gathers()

        # ---------------- final: out[b, t] = b2 + W2row . cat(x, uemb) ------
        xT_ps = ps_m.tile([BPC, FC], F32, tag="m")
        nc.tensor.transpose(xT_ps[:], x[:], ident[:FC, :FC])
        nc.vector.tensor_copy(xcat[:, 0:FC], xT_ps[:])

        out_sb = const.tile([BPC, T], F32)
        for t in range(T):
            dummy = scrp.tile([BPC, FC + D], F32, tag="fdum")
            acc = scrp.tile([BPC, 1], F32, tag="facc")
            nc.vector.scalar_tensor_tensor(
                out=dummy[:],
                in0=w2g[t][:],
                scalar=1.0,
                in1=xcat[:],
                op0=Mult,
                op1=Mult,
                accum_out=acc[:],
            )
            nc.vector.tensor_tensor(
                out=out_sb[:, t : t + 1], in0=acc[:], in1=b2g[:, t : t + 1], op=Add
            )
        nc.sync.dma_start(out=out_d[:, :], in_=out_sb[:])

    nc.finalize()
    return nc


def get_program(cfg=None):
    cfg = dict(CFG if cfg is None else cfg)
    key = tuple(sorted(cfg.items()))
    if key not in _PROG_CACHE:
        _PROG_CACHE[key] = _build_program(cfg)
    return _PROG_CACHE[key]


def make_in_maps(inputs):
    """Shard the full-problem inputs into 8 per-core input maps."""
    seq = np.asarray(inputs["seq_var"]).astype(np.int32)
    usr = np.asarray(inputs["user_var"]).astype(np.int32).reshape(B_FULL, 1)
    itm = np.asarray(inputs["item_var"]).astype(np.int32).reshape(B_FULL, T)
    shared = dict(
        item_emb=np.ascontiguousarray(np.asarray(inputs["item_emb"], np.float32)),
        user_emb=np.ascontiguousarray(np.asarray(inputs["user_emb"], np.float32)),
        W2=np.ascontiguousarray(np.asarray(inputs["W2"], np.float32)),
        b2=np.ascontiguousarray(np.asarray(inputs["b2"], np.float32).reshape(-1, 1)),
        W1=np.ascontiguousarray(np.asarray(inputs["W1"], np.float32)),
        b1=np.ascontiguousarray(np.asarray(inputs["b1"], np.float32).reshape(FC, 1)),
        Wf2=np.ascontiguousarray(np.asarray(inputs["Wf2"], np.float32)),
        bf2=np.ascontiguousarray(np.asarray(inputs["bf2"], np.float32).reshape(FC, 1)),
    )
    in_maps = []
    for c in range(N_CORES):
        rows = slice(c * BPC, (c + 1) * BPC)
        flat = seq[rows].reshape(NROW)               # (b*L + l) order
        seq_pm = np.ascontiguousarray(flat.reshape(NTILE, 128).T)  # [128, 16]
        in_maps.append(
            dict(
                shared,
                seq_idx=seq_pm,
                user_idx=np.ascontiguousarray(usr[rows]),
                item_idx=np.ascontiguousarray(itm[rows]),
            )
        )
    return in_maps


def run_sharded(inputs, cfg=None, trace=False, **kwargs):
    nc = get_program(cfg)
    in_maps = make_in_maps(inputs)
    res = run_bass_kernel_spmd(nc, in_maps, list(range(N_CORES)), trace=trace, **kwargs)
    out = np.concatenate([r["out"] for r in res.results], axis=0)
    return out, res


def kernel(**inputs) -> np.ndarray:
    out, _ = run_sharded(inputs)
    return out
